# revision 65
# baseline (speedup 1.0000x reference)
"""Trainium2 Bass kernel for nn_AutoPruneNet (MLP policy/baseline heads + sampling).

Math (per row r of TB = T*B rows):
    h1 = relu(x @ W1.T + b1)            x: [512], h1: [400]
    h2 = relu(h1 @ W2.T + b2)           h2: [300]
    core = [h2, clip(reward,-1,1), last_action]   [302]
    pl = sigmoid(core @ Wp.T + bp)      [2]  (mu, sigma)
    baseline = core @ Wb.T + bb         [1]
    action = pl0 + pl1 * eps
    out[r] = [pl0, pl1, baseline, action]

Distribution: pure data parallel, TB rows split contiguously across 8 cores
(16384 rows each); weights replicated.

Device layout: activations stay feature-major ("transposed"): [feature, row],
so the contraction dim of every matmul sits on SBUF partitions and no on-chip
transposes are needed. The host pre-transposes the frame once and the output
back once.

All three layers run in fp8e4 with MatmulPerfMode.DoubleRow (2 contraction
rows per PE cell -> 256-deep contraction per instruction), halving the matmul
stream count vs bf16: per 512-row tile fc1 is 8 streams, fc2 is 6, heads 2.
fp8e4 subnormal loss on the tiny MLP weights is avoided by power-of-2 weight
scaling on the host (W1*32, W2*32, Wh*16); the scales ride along the
activations (h1' = 32*relu(h1) fp8, h2' = 64*relu(h2) fp8 via the fc2 relu's
scale=1/16, head psum = 1024*head_pre undone by the psum-evacuation copy's
scale=1/1024), with biases pre-scaled so no extra device ops are needed.

Feature padding: h1 is padded 400->512 (zero W1 columns) so fc1 emits four
uniform 128-wide chunks forming two [128, 2, NT] fp8 DoubleRow pairs; fc2
output is padded 300->320 with chunks {128, 128, 64} so fc2's first two
chunks form the first head DoubleRow pair and the third chunk (44 real rows
+ zeros) pairs with a plane holding [clip(reward); last_action; ones]
(host-prepared) at partitions 32..34 — the ones row folds every head bias
into the head matmul. Head outputs land on psum partitions 0/1/2 (of 16).

Epilogue is transposed: per group of 4 tiles, ACT evacuates the [16, NT]
head psums into a [16, 4*NT] staging tile and one XBAR DMA transposes it to
rows-as-partitions layout (row 512t + 128c + p at [p, 4t+c, :]), so
sigmoid/mul/add run with free-dim 16-32 instead of 512 and the output planes
accumulate in SBUF, shipped by three contiguous DMAs at the end. eps arrives
pre-transposed the same way (host-side, free).

The software pipeline is two tiles deep — emit order fc1(t), heads(t-2),
fc2(t-1) — so the in-order PE stream never waits on the current tile's
relus; the XBAR (Sync, one tile after the last head copy) and the epilogue
ops (one tile later still) never block xt loads or fc2 relus."""
import sys
import types

import numpy as np
import ml_dtypes

import concourse.bacc as bacc
import concourse.bass as bass
import concourse.mybir as mybir
import concourse.tile as tile
from concourse.bass import ds, ts
from concourse.bass_utils import run_bass_kernel_spmd


def _install_ntff_hook_shim():
    """Provide the optional antenv.axon_hooks module if the image lacks it,
    so a BASS_TRACE env var in the caller can't crash run_bass_kernel_spmd.
    Registers the real NTFF profile hook when the axon .so supports it."""
    try:
        import antenv.axon_hooks  # noqa: F401
        return
    except Exception:
        pass
    try:
        import antenv
    except Exception:
        return
    mod = types.ModuleType("antenv.axon_hooks")
    state = {"hook": None}
    mod.set_axon_ntff_profile_hook = lambda h: state.__setitem__("hook", h)
    mod.get_axon_ntff_profile_hook = lambda: state["hook"]
    sys.modules["antenv.axon_hooks"] = mod
    antenv.axon_hooks = mod
    try:
        from trn_agent_boot.trn_boot import _ntff_profile_via_ctypes
        mod.set_axon_ntff_profile_hook(
            _ntff_profile_via_ctypes('/opt/axon/libaxon_pjrt.so'))
    except Exception:
        pass


_install_ntff_hook_shim()

BF16 = ml_dtypes.bfloat16
FP8 = ml_dtypes.float8_e4m3   # TRN fp8e4 flavor (max +-240)

N_CORES = 8
T, B, OBS = 64, 2048, 512
H1, H2 = 400, 300
TB = T * B
R = TB // N_CORES       # rows per core
NT = 512                # rows per row-tile (matmul moving dim)
OG = 8                  # row-tiles per output-DMA group
NJ = R // 128           # transposed-layout columns per core

S1 = 32.0               # scale on h1' = S1 * relu(h1)  (fp8 storage)
S2 = 1024.0             # scale of the fc2 psum (S2 * h2_pre)
S2P = 64.0              # scale on h2' = S2P * relu(h2) (fp8 storage)
SH = 1024.0             # scale of the head psum (SH * head_pre)

F32 = mybir.dt.float32
BF = mybir.dt.bfloat16
F8 = mybir.dt.float8e4
AF = mybir.ActivationFunctionType
ALU = mybir.AluOpType
DR = mybir.MatmulPerfMode.DoubleRow

# fc2 output (h2) chunking: {128, 128, 64}; chunk 2 covers h2[256:320]
# (300:320 zero-padded)
M2 = [(0, 128), (128, 128), (256, 64)]


def build_bass(rows: int):
    """Build the per-core Bass program for `rows` rows (rows % (NT*OG) == 0)."""
    assert rows % (NT * OG) == 0
    n_tiles = rows // NT
    n_groups = n_tiles // OG
    nj = rows // 128

    nc = bacc.Bacc("TRN2", target_bir_lowering=False, debug=False)

    xt_d = nc.dram_tensor("xt", [128, 4, rows], F8, kind="ExternalInput")
    rwla_d = nc.dram_tensor("rwla", [3, rows], F8, kind="ExternalInput")
    eps_d = nc.dram_tensor("eps", [128, nj], F32, kind="ExternalInput")
    w1_d = nc.dram_tensor("w1", [128, 4, 512], F8, kind="ExternalInput")
    w2_d = nc.dram_tensor("w2", [128, 4, 320], F8, kind="ExternalInput")
    wha_d = nc.dram_tensor("wha", [128, 2, 16], F8, kind="ExternalInput")
    whb_d = nc.dram_tensor("whb", [64, 2, 16], F8, kind="ExternalInput")
    b1_d = nc.dram_tensor("b1", [128, 4], F32, kind="ExternalInput")
    b2_d = nc.dram_tensor("b2", [128, 3], F32, kind="ExternalInput")
    pl_d = nc.dram_tensor("pl", [128, nj, 2], BF, kind="ExternalOutput")
    act_d = nc.dram_tensor("act", [128, nj], BF, kind="ExternalOutput")
    bl_d = nc.dram_tensor("bl", [128, nj], BF, kind="ExternalOutput")

    with tile.TileContext(nc) as tc:
        with (
            tc.tile_pool(name="w", bufs=1) as wpool,
            tc.tile_pool(name="x", bufs=6) as xpool,
            tc.tile_pool(name="h1", bufs=4) as h1pool,
            tc.tile_pool(name="core", bufs=6) as cpool,
            tc.tile_pool(name="s", bufs=4) as spool,
            tc.tile_pool(name="ob", bufs=3) as opool,
            tc.tile_pool(name="ps1", bufs=4, space="PSUM") as ppool1,
            tc.tile_pool(name="ps2", bufs=2, space="PSUM") as ppool2,
            tc.tile_pool(name="ps3", bufs=2, space="PSUM") as ppool3,
        ):
            # w1 is on the critical path to the first matmul: split it
            # across two DMA queues
            w1_sb = wpool.tile([128, 4, 512], F8, tag="w1")
            nc.scalar.dma_start(w1_sb[:, 0:2, :], w1_d[:, 0:2, :])
            nc.scalar.dma_start(w1_sb[:, 2:4, :], w1_d[:, 2:4, :])
            b1_sb = wpool.tile([128, 4, 1], F32, tag="b1")
            nc.scalar.dma_start(b1_sb[:], b1_d[:])
            w2_sb = wpool.tile([128, 4, 320], F8, tag="w2")
            nc.scalar.dma_start(w2_sb[:], w2_d[:])
            b2_sb = wpool.tile([128, 3, 1], F32, tag="b2")
            nc.scalar.dma_start(b2_sb[:], b2_d[:])
            wha_sb = wpool.tile([128, 2, 16], F8, tag="wha")
            nc.scalar.dma_start(wha_sb[:], wha_d[:])
            whb_sb = wpool.tile([64, 2, 16], F8, tag="whb")
            nc.scalar.dma_start(whb_sb[:], whb_d[:])

            # Global transposed-epilogue accumulators (tiny: <= 1KB/partition)
            epsT_sb = wpool.tile([128, nj], F32, tag="epsT")
            nc.scalar.dma_start(epsT_sb[:], eps_d[:])
            plT = wpool.tile([128, nj, 2], BF, tag="plT")
            actT = wpool.tile([128, nj], BF, tag="actT")
            blT = wpool.tile([128, nj], BF, tag="blT")

            groups = {}     # g -> (sig16g, cB)
            sig16gs = {}    # g -> completed head-psum staging tile
            xstages = {}    # g -> xstage tile (XBAR output)
            h1s = {}        # t -> (hA, hB) fc1 output pairs
            cAs = {}        # t -> cA head pair (fc2 chunks 0/1)

            def prep_group(g):
                # prepared two tiles ahead of the group boundary so the rwla
                # DMA issue and the cB memset never sit in front of relus on
                # the Scalar/GpSimd streams when the boundary arrives
                sig16g = opool.tile([16, OG * NT], BF, tag="sig16g")
                # cB: head DoubleRow pair B: plane 0 = h2'[256:320]
                # (300:320 zero), plane 1 = zeros with [cr; la; ones]
                # at partitions 32..34
                cB = opool.tile([64, 2, OG * NT], F8, tag="cB")
                nc.gpsimd.memset(cB[0:64, 1, :], 0.0)
                nc.scalar.dma_start(cB[32:35, 1, :],
                                    rwla_d[:, ts(g, OG * NT)])
                groups[g] = (sig16g, cB)

            def emit_fc1(t):
                xt_t = xpool.tile([128, 4, NT], F8, tag="xt")
                if t == 0:
                    # split the first load across two queues: the first
                    # DoubleRow pair only needs planes 0:2, so the opening
                    # matmul starts ~2us earlier
                    nc.sync.dma_start(xt_t[:, 0:2, :], xt_d[:, 0:2, ts(t, NT)])
                    nc.sync.dma_start(xt_t[:, 2:4, :], xt_d[:, 2:4, ts(t, NT)])
                else:
                    nc.sync.dma_start(xt_t[:], xt_d[:, :, ts(t, NT)])
                # fc1: h1'T = S1*relu(h1_pre) in two fp8 DoubleRow pair
                # tiles [128, 2, NT]: hA = feats 0:256, hB = 256:512
                # (400:512 zero via zero-padded W1 columns)
                hA = h1pool.tile([128, 2, NT], F8, tag="h1a")
                hB = h1pool.tile([128, 2, NT], F8, tag="h1b")
                for m in range(4):
                    ps = ppool1.tile([128, NT], F32, tag="ps1")
                    nc.tensor.matmul(
                        ps[:], w1_sb[:, 0:2, ds(m * 128, 128)],
                        xt_t[:, 0:2, :],
                        start=True, stop=False, perf_mode=DR)
                    nc.tensor.matmul(
                        ps[:], w1_sb[:, 2:4, ds(m * 128, 128)],
                        xt_t[:, 2:4, :],
                        start=False, stop=True, perf_mode=DR)
                    ht = (hA, hB)[m // 2]
                    # relu(psum + S1*b1): m<3 on DVE ((in + bias) max 0);
                    # the mostly-pad m=3 chunk goes to ACT to shorten the
                    # DVE chain that gates fc2's second DoubleRow stream
                    if m < 3:
                        nc.vector.tensor_scalar(
                            ht[:, m % 2, :], ps[:], b1_sb[:, m, :], 0.0,
                            ALU.add, ALU.max)
                    else:
                        nc.scalar.activation(ht[:, m % 2, :], ps[:], AF.Relu,
                                             bias=b1_sb[:, m, :])
                h1s[t] = (hA, hB)

            def emit_fc2(t):
                # fc2: h2' = S2P*relu(h2_pre) in fp8, chunks {128, 128, 64};
                # the m=2 chunk goes first so the cB assembly finishes before
                # the head matmuls consume it
                hA, hB = h1s.pop(t)
                g, ti = divmod(t, OG)
                cB = groups[g][1]
                cA = cpool.tile([128, 2, NT], F8, tag="cA")
                for m in (2, 0, 1):
                    m0, mw = M2[m]
                    ps2 = ppool2.tile([mw, NT], F32, tag="ps2")
                    nc.tensor.matmul(
                        ps2[:], w2_sb[:, 0:2, ds(m0, mw)], hA[:],
                        start=True, stop=False, perf_mode=DR)
                    nc.tensor.matmul(
                        ps2[:], w2_sb[:, 2:4, ds(m0, mw)], hB[:],
                        start=False, stop=True, perf_mode=DR)
                    if m < 2:
                        nc.scalar.activation(cA[:, m, :], ps2[:], AF.Relu,
                                             bias=b2_sb[:, m, :],
                                             scale=S2P / S2)
                    else:
                        nc.scalar.activation(cB[0:mw, 0, ts(ti, NT)],
                                             ps2[:], AF.Relu,
                                             bias=b2_sb[0:mw, m, :],
                                             scale=S2P / S2)
                cAs[t] = cA

            def emit_heads(t):
                g, ti = divmod(t, OG)
                sig16g, cB = groups[g]
                cA = cAs.pop(t)
                # heads (+ biases via the ones row), fp8 DoubleRow: psum
                # partitions 0=mu_pre, 1=sigma_pre, 2=baseline_pre (*SH)
                psh = ppool3.tile([16, NT], F32, tag="ps3")
                nc.tensor.matmul(psh[:], whb_sb[:], cB[:, :, ts(ti, NT)],
                                 start=True, stop=False, perf_mode=DR)
                nc.tensor.matmul(psh[:], wha_sb[:], cA[:],
                                 start=False, stop=True, perf_mode=DR)
                # ACT evacuates the head psum to SBUF (bf16; DMA cannot read
                # PSUM) into the group staging tile, undoing the SH scale
                nc.scalar.activation(sig16g[:, ts(ti, NT)], psh[:], AF.Copy,
                                     scale=1.0 / SH)
                if ti == OG - 1:
                    sig16gs[g] = sig16g
                    del groups[g]

            def emit_group_xbar(g):
                # transposed epilogue, once per group: one XBAR DMA transposes
                # the staged head outputs to rows-as-partitions layout (row
                # 512t + 128c + p at xstage[p, 4t+c, :]), so sigmoid/mul/add
                # run with free-dim 16-32 instead of 512. Issued on Sync one
                # tile after the last head-psum copy, so its wait is already
                # satisfied and it never delays xt loads or fc2 relus.
                xstage = opool.tile([128, 4 * OG, 16], BF, tag="xstage")
                nc.sync.dma_start_transpose(xstage[:], sig16gs.pop(g))
                xstages[g] = xstage

            def emit_group_epilogue(g):
                gsl = ds(g * 4 * OG, 4 * OG)
                xstage = xstages.pop(g)
                nc.scalar.activation(plT[:, gsl, :], xstage[:, :, 0:2],
                                     AF.Sigmoid)
                se = spool.tile([128, 4 * OG], F32, tag="se")
                nc.vector.tensor_mul(se[:], plT[:, gsl, 1], epsT_sb[:, gsl])
                nc.vector.tensor_add(actT[:, gsl], plT[:, gsl, 0], se[:])
                nc.vector.tensor_copy(blT[:, gsl], xstage[:, :, 2])

            prep_group(0)
            for t in range(n_tiles + 2):
                if t % OG == 2 and t >= OG + 2:
                    emit_group_xbar(t // OG - 1)
                if t % OG == 0 and t >= 2 * OG:
                    emit_group_epilogue(t // OG - 2)
                if t % OG == 2 and t // OG + 1 < n_groups:
                    prep_group(t // OG + 1)
                if t < n_tiles:
                    emit_fc1(t)
                if 0 <= t - 2:
                    emit_heads(t - 2)
                if 0 <= t - 1 < n_tiles:
                    emit_fc2(t - 1)

            emit_group_xbar(n_groups - 1)
            emit_group_epilogue(n_groups - 1)
            # ship the accumulated output planes once, fully contiguous on
            # both sides, spread across both DMA-capable engines
            nc.sync.dma_start(pl_d[:], plT[:])
            nc.scalar.dma_start(act_d[:], actT[:])
            nc.sync.dma_start(bl_d[:], blT[:])

    nc.compile()
    return nc


def host_prep(frame, reward, last_action, eps, W1, b1, W2, b2, Wp, bp, Wb, bb,
              rows=R, n_cores=N_CORES):
    """Shard + lay out inputs for the device program. Returns in_maps."""
    frame = np.asarray(frame, np.float32).reshape(TB, OBS)
    reward = np.asarray(reward, np.float32).reshape(TB)
    la = np.asarray(last_action).reshape(TB).astype(FP8)
    eps = np.asarray(eps, np.float32).reshape(TB)

    W1 = np.asarray(W1, np.float32)
    W2 = np.asarray(W2, np.float32)
    b1 = np.asarray(b1, np.float32)
    b2 = np.asarray(b2, np.float32)
    Wp = np.asarray(Wp, np.float32)
    bp = np.asarray(bp, np.float32)
    Wb = np.asarray(Wb, np.float32)
    bb = np.asarray(bb, np.float32)

    # W1T scaled by S1, padded 400 -> 512 output features, fp8:
    # w1[p, b, o] = S1 * W1[o, 128b+p]
    w1t = np.zeros((512, 512), np.float32)
    w1t[:, 0:400] = S1 * W1.T
    w1_h = np.ascontiguousarray(
        w1t.reshape(4, 128, 512).transpose(1, 0, 2)).astype(FP8)
    # W2T scaled by S2/S1, padded [400->512, 300->320], fp8
    w2t = np.zeros((512, 320), np.float32)
    w2t[0:400, 0:300] = (S2 / S1) * W2.T
    w2_h = np.ascontiguousarray(
        w2t.reshape(4, 128, 320).transpose(1, 0, 2)).astype(FP8)
    # head weights (fp8, scaled by SH/S2P on h2 rows): columns 0/1/2 hold
    # (mu, sigma, baseline). Pair A planes = h2 feats 0:128 / 128:256;
    # pair B plane 0 = feats 256:320 (pad), plane 1 = [cr; la; ones]
    # weights at rows 32..34 with the ones row carrying the head biases
    # (scaled by SH since cr/la/ones are unscaled on the device)
    Wh16 = np.zeros((302, 16), np.float32)
    Wh16[:, 0] = Wp[0]
    Wh16[:, 1] = Wp[1]
    Wh16[:, 2] = Wb[0]
    wha_h = np.zeros((128, 2, 16), np.float32)
    wha_h[:, 0, :] = (SH / S2P) * Wh16[0:128]
    wha_h[:, 1, :] = (SH / S2P) * Wh16[128:256]
    wha_h = wha_h.astype(FP8)
    whb_h = np.zeros((64, 2, 16), np.float32)
    whb_h[0:44, 0, :] = (SH / S2P) * Wh16[256:300]
    whb_h[32:34, 1, :] = SH * Wh16[300:302]
    whb_h[34, 1, 0] = SH * bp[0]
    whb_h[34, 1, 1] = SH * bp[1]
    whb_h[34, 1, 2] = SH * bb[0]
    whb_h = whb_h.astype(FP8)
    b1p = np.zeros(512, np.float32)
    b1p[0:400] = S1 * b1
    b1_h = np.ascontiguousarray(b1p.reshape(4, 128).T)
    b2_h = np.zeros((128, 3), np.float32)
    b2_h[0:128, 0] = S2P * b2[0:128]
    b2_h[0:128, 1] = S2P * b2[128:256]
    b2_h[0:44, 2] = S2P * b2[256:300]

    cr = np.clip(reward, -1.0, 1.0).astype(FP8)
    ones = np.ones(rows, FP8)

    in_maps = []
    for c in range(n_cores):
        sl = slice(c * rows, (c + 1) * rows)
        xt = np.ascontiguousarray(
            frame[sl].T.reshape(4, 128, rows).transpose(1, 0, 2)).astype(FP8)
        rwla = np.stack([cr[sl], la[sl], ones], axis=0)
        # transposed row layout: row 512*t + 128*c + p sits at column 4*t + c
        # of partition p (the XBAR transpose's 128x16-tile output ordering)
        in_maps.append({
            "xt": xt,
            "rwla": rwla,
            "eps": np.ascontiguousarray(
                eps[sl].reshape(-1, 4, 128).transpose(2, 0, 1).reshape(
                    128, -1)),
            "w1": w1_h, "w2": w2_h, "wha": wha_h, "whb": whb_h,
            "b1": b1_h, "b2": b2_h,
        })
    return in_maps


def assemble_out(per_core_results):
    """Per-core dict of transposed-layout planes (row 512*t + 128*c + p at
    column 4*t + c of partition p): pl [128, nj, 2], act [128, nj],
    bl [128, nj] -> [T, B, 4]."""
    outs = []
    for r in per_core_results:
        pl = np.asarray(r["pl"]).astype(np.float32)      # [128, nj, 2]
        act = np.asarray(r["act"]).astype(np.float32)    # [128, nj]
        bl = np.asarray(r["bl"]).astype(np.float32)      # [128, nj]
        o = np.stack([pl[:, :, 0], pl[:, :, 1], bl, act], axis=-1)
        # [p, 4t+c, ch] -> rows 512t + 128c + p
        o = o.reshape(128, -1, 4, 4).transpose(1, 2, 0, 3).reshape(-1, B, 4)
        outs.append(o)
    return np.ascontiguousarray(np.concatenate(outs, axis=0))


_NC_CACHE = {}


def kernel(**inputs) -> np.ndarray:
    in_maps = host_prep(**inputs)
    if R not in _NC_CACHE:
        _NC_CACHE[R] = build_bass(R)
    nc = _NC_CACHE[R]
    res = run_bass_kernel_spmd(nc, in_maps, core_ids=list(range(N_CORES)))
    return assemble_out([res.results[c] for c in range(N_CORES)])


# revision 66
# speedup vs baseline: 1.2194x; 1.2194x over previous
"""Trainium2 Bass kernel for nn_AutoPruneNet (MLP policy/baseline heads + sampling).

Math (per row r of TB = T*B rows):
    h1 = relu(x @ W1.T + b1)            x: [512], h1: [400]
    h2 = relu(h1 @ W2.T + b2)           h2: [300]
    core = [h2, clip(reward,-1,1), last_action]   [302]
    pl = sigmoid(core @ Wp.T + bp)      [2]  (mu, sigma)
    baseline = core @ Wb.T + bb         [1]
    action = pl0 + pl1 * eps
    out[r] = [pl0, pl1, baseline, action]

Distribution: pure data parallel, TB rows split contiguously across 8 cores
(16384 rows each); weights replicated.

Device layout: activations stay feature-major ("transposed"): [feature, row],
so the contraction dim of every matmul sits on SBUF partitions and no on-chip
transposes are needed. The host pre-transposes the frame once and the output
back once.

All three layers run in fp8e4 with MatmulPerfMode.DoubleRow (2 contraction
rows per PE cell -> 256-deep contraction per instruction), halving the matmul
stream count vs bf16: per 512-row tile fc1 is 8 streams, fc2 is 6, heads 2.
fp8e4 subnormal loss on the tiny MLP weights is avoided by power-of-2 weight
scaling on the host (W1*32, W2*32, Wh*16); the scales ride along the
activations (h1' = 32*relu(h1) fp8, h2' = 64*relu(h2) fp8 via the fc2 relu's
scale=1/16, head psum = 1024*head_pre undone by the psum-evacuation copy's
scale=1/1024), with biases pre-scaled so no extra device ops are needed.

Feature padding: h1 is padded 400->512 (zero W1 columns) so fc1 emits four
uniform 128-wide chunks forming two [128, 2, NT] fp8 DoubleRow pairs; fc2
output is padded 300->320 with chunks {128, 128, 64} so fc2's first two
chunks form the first head DoubleRow pair and the third chunk (44 real rows
+ zeros) pairs with a plane holding [clip(reward); last_action; ones]
(host-prepared) at partitions 32..34 — the ones row folds every head bias
into the head matmul. Head outputs land on psum partitions 0/1/2 (of 16).

Epilogue is transposed: per group of 4 tiles, ACT evacuates the [16, NT]
head psums into a [16, 4*NT] staging tile and one XBAR DMA transposes it to
rows-as-partitions layout (row 512t + 128c + p at [p, 4t+c, :]), so
sigmoid/mul/add run with free-dim 16-32 instead of 512 and the output planes
accumulate in SBUF, shipped by three contiguous DMAs at the end. eps arrives
pre-transposed the same way (host-side, free).

The software pipeline is two tiles deep — emit order fc1(t), heads(t-2),
fc2(t-1) — so the in-order PE stream never waits on the current tile's
relus; the XBAR (Sync, one tile after the last head copy) and the epilogue
ops (one tile later still) never block xt loads or fc2 relus."""
import sys
import types

import numpy as np
import ml_dtypes

import concourse.bacc as bacc
import concourse.bass as bass
import concourse.mybir as mybir
import concourse.tile as tile
from concourse.bass import ds, ts
from concourse.bass_utils import run_bass_kernel_spmd


def _install_ntff_hook_shim():
    """Provide the optional antenv.axon_hooks module if the image lacks it,
    so a BASS_TRACE env var in the caller can't crash run_bass_kernel_spmd.
    Registers the real NTFF profile hook when the axon .so supports it."""
    try:
        import antenv.axon_hooks  # noqa: F401
        return
    except Exception:
        pass
    try:
        import antenv
    except Exception:
        return
    mod = types.ModuleType("antenv.axon_hooks")
    state = {"hook": None}
    mod.set_axon_ntff_profile_hook = lambda h: state.__setitem__("hook", h)
    mod.get_axon_ntff_profile_hook = lambda: state["hook"]
    sys.modules["antenv.axon_hooks"] = mod
    antenv.axon_hooks = mod
    try:
        from trn_agent_boot.trn_boot import _ntff_profile_via_ctypes
        mod.set_axon_ntff_profile_hook(
            _ntff_profile_via_ctypes('/opt/axon/libaxon_pjrt.so'))
    except Exception:
        pass


_install_ntff_hook_shim()

BF16 = ml_dtypes.bfloat16
FP8 = ml_dtypes.float8_e4m3   # TRN fp8e4 flavor (max +-240)

N_CORES = 8
T, B, OBS = 64, 2048, 512
H1, H2 = 400, 300
TB = T * B
R = TB // N_CORES       # rows per core
NT = 512                # rows per row-tile (matmul moving dim)
OG = 8                  # row-tiles per output-DMA group
NJ = R // 128           # transposed-layout columns per core

S1 = 32.0               # scale on h1' = S1 * relu(h1)  (fp8 storage)
S2 = 1024.0             # scale of the fc2 psum (S2 * h2_pre)
S2P = 64.0              # scale on h2' = S2P * relu(h2) (fp8 storage)
SH = 1024.0             # scale of the head psum (SH * head_pre)

F32 = mybir.dt.float32
BF = mybir.dt.bfloat16
F8 = mybir.dt.float8e4
AF = mybir.ActivationFunctionType
ALU = mybir.AluOpType
DR = mybir.MatmulPerfMode.DoubleRow

# fc2 output (h2) chunking: {128, 128, 64}; chunk 2 covers h2[256:320]
# (300:320 zero-padded)
M2 = [(0, 128), (128, 128), (256, 64)]


def build_bass(rows: int):
    """Build the per-core Bass program for `rows` rows (rows % (NT*OG) == 0)."""
    assert rows % (NT * OG) == 0
    n_tiles = rows // NT
    n_groups = n_tiles // OG
    nj = rows // 128

    nc = bacc.Bacc("TRN2", target_bir_lowering=False, debug=False)

    xt_d = nc.dram_tensor("xt", [128, 4, rows], F8, kind="ExternalInput")
    rwla_d = nc.dram_tensor("rwla", [3, rows], F8, kind="ExternalInput")
    eps_d = nc.dram_tensor("eps", [128, nj], F32, kind="ExternalInput")
    w1_d = nc.dram_tensor("w1", [128, 4, 512], F8, kind="ExternalInput")
    w2_d = nc.dram_tensor("w2", [128, 4, 320], F8, kind="ExternalInput")
    wha_d = nc.dram_tensor("wha", [128, 2, 16], F8, kind="ExternalInput")
    whb_d = nc.dram_tensor("whb", [64, 2, 16], F8, kind="ExternalInput")
    b1_d = nc.dram_tensor("b1", [128, 4], F32, kind="ExternalInput")
    b2_d = nc.dram_tensor("b2", [128, 3], F32, kind="ExternalInput")
    pl_d = nc.dram_tensor("pl", [128, nj, 2], BF, kind="ExternalOutput")
    act_d = nc.dram_tensor("act", [128, nj], BF, kind="ExternalOutput")
    bl_d = nc.dram_tensor("bl", [128, nj], BF, kind="ExternalOutput")

    with tile.TileContext(nc) as tc:
        with (
            tc.tile_pool(name="w", bufs=1) as wpool,
            tc.tile_pool(name="x", bufs=6) as xpool,
            tc.tile_pool(name="h1", bufs=4) as h1pool,
            tc.tile_pool(name="core", bufs=6) as cpool,
            tc.tile_pool(name="s", bufs=4) as spool,
            tc.tile_pool(name="ob", bufs=3) as opool,
            tc.tile_pool(name="ps1", bufs=4, space="PSUM") as ppool1,
            tc.tile_pool(name="ps2", bufs=2, space="PSUM") as ppool2,
            tc.tile_pool(name="ps3", bufs=2, space="PSUM") as ppool3,
        ):
            # w1 is on the critical path to the first matmul: split it
            # across two DMA queues
            w1_sb = wpool.tile([128, 4, 512], F8, tag="w1")
            nc.scalar.dma_start(w1_sb[:, 0:2, :], w1_d[:, 0:2, :])
            nc.scalar.dma_start(w1_sb[:, 2:4, :], w1_d[:, 2:4, :])
            b1_sb = wpool.tile([128, 4, 1], F32, tag="b1")
            nc.scalar.dma_start(b1_sb[:], b1_d[:])
            w2_sb = wpool.tile([128, 4, 320], F8, tag="w2")
            nc.scalar.dma_start(w2_sb[:], w2_d[:])
            b2_sb = wpool.tile([128, 3, 1], F32, tag="b2")
            nc.scalar.dma_start(b2_sb[:], b2_d[:])
            wha_sb = wpool.tile([128, 2, 16], F8, tag="wha")
            nc.scalar.dma_start(wha_sb[:], wha_d[:])
            whb_sb = wpool.tile([64, 2, 16], F8, tag="whb")
            nc.scalar.dma_start(whb_sb[:], whb_d[:])

            # Global transposed-epilogue accumulators (tiny: <= 1KB/partition)
            epsT_sb = wpool.tile([128, nj], F32, tag="epsT")
            nc.scalar.dma_start(epsT_sb[:], eps_d[:])
            plT = wpool.tile([128, nj, 2], BF, tag="plT")
            actT = wpool.tile([128, nj], BF, tag="actT")
            blT = wpool.tile([128, nj], BF, tag="blT")

            groups = {}     # g -> (sig16g, cB)
            sig16gs = {}    # g -> completed head-psum staging tile
            xstages = {}    # g -> xstage tile (XBAR output)
            h1s = {}        # t -> (hA, hB) fc1 output pairs
            cAs = {}        # t -> cA head pair (fc2 chunks 0/1)

            def prep_group(g):
                # prepared two tiles ahead of the group boundary so the rwla
                # DMA issue and the cB memset never sit in front of relus on
                # the Scalar/GpSimd streams when the boundary arrives
                sig16g = opool.tile([16, OG * NT], BF, tag="sig16g")
                # cB: head DoubleRow pair B: plane 0 = h2'[256:320]
                # (300:320 zero), plane 1 = zeros with [cr; la; ones]
                # at partitions 32..34
                cB = opool.tile([64, 2, OG * NT], F8, tag="cB")
                nc.gpsimd.memset(cB[0:64, 1, :], 0.0)
                nc.scalar.dma_start(cB[32:35, 1, :],
                                    rwla_d[:, ts(g, OG * NT)])
                groups[g] = (sig16g, cB)

            def emit_fc1(t):
                xt_t = xpool.tile([128, 4, NT], F8, tag="xt")
                nc.sync.dma_start(xt_t[:], xt_d[:, :, ts(t, NT)])
                # fc1: h1'T = S1*relu(h1_pre) in two fp8 DoubleRow pair
                # tiles [128, 2, NT]: hA = feats 0:256, hB = 256:512
                # (400:512 zero via zero-padded W1 columns)
                hA = h1pool.tile([128, 2, NT], F8, tag="h1a")
                hB = h1pool.tile([128, 2, NT], F8, tag="h1b")
                for m in range(4):
                    ps = ppool1.tile([128, NT], F32, tag="ps1")
                    nc.tensor.matmul(
                        ps[:], w1_sb[:, 0:2, ds(m * 128, 128)],
                        xt_t[:, 0:2, :],
                        start=True, stop=False, perf_mode=DR)
                    nc.tensor.matmul(
                        ps[:], w1_sb[:, 2:4, ds(m * 128, 128)],
                        xt_t[:, 2:4, :],
                        start=False, stop=True, perf_mode=DR)
                    ht = (hA, hB)[m // 2]
                    # relu(psum + S1*b1) on DVE: (in + bias) max 0
                    nc.vector.tensor_scalar(
                        ht[:, m % 2, :], ps[:], b1_sb[:, m, :], 0.0,
                        ALU.add, ALU.max)
                h1s[t] = (hA, hB)

            def emit_fc2(t):
                # fc2: h2' = S2P*relu(h2_pre) in fp8, chunks {128, 128, 64};
                # the m=2 chunk goes first so the cB assembly finishes before
                # the head matmuls consume it
                hA, hB = h1s.pop(t)
                g, ti = divmod(t, OG)
                cB = groups[g][1]
                cA = cpool.tile([128, 2, NT], F8, tag="cA")
                for m in (2, 0, 1):
                    m0, mw = M2[m]
                    ps2 = ppool2.tile([mw, NT], F32, tag="ps2")
                    nc.tensor.matmul(
                        ps2[:], w2_sb[:, 0:2, ds(m0, mw)], hA[:],
                        start=True, stop=False, perf_mode=DR)
                    nc.tensor.matmul(
                        ps2[:], w2_sb[:, 2:4, ds(m0, mw)], hB[:],
                        start=False, stop=True, perf_mode=DR)
                    if m < 2:
                        nc.scalar.activation(cA[:, m, :], ps2[:], AF.Relu,
                                             bias=b2_sb[:, m, :],
                                             scale=S2P / S2)
                    else:
                        nc.scalar.activation(cB[0:mw, 0, ts(ti, NT)],
                                             ps2[:], AF.Relu,
                                             bias=b2_sb[0:mw, m, :],
                                             scale=S2P / S2)
                cAs[t] = cA

            def emit_heads(t):
                g, ti = divmod(t, OG)
                sig16g, cB = groups[g]
                cA = cAs.pop(t)
                # heads (+ biases via the ones row), fp8 DoubleRow: psum
                # partitions 0=mu_pre, 1=sigma_pre, 2=baseline_pre (*SH)
                psh = ppool3.tile([16, NT], F32, tag="ps3")
                nc.tensor.matmul(psh[:], whb_sb[:], cB[:, :, ts(ti, NT)],
                                 start=True, stop=False, perf_mode=DR)
                nc.tensor.matmul(psh[:], wha_sb[:], cA[:],
                                 start=False, stop=True, perf_mode=DR)
                # ACT evacuates the head psum to SBUF (bf16; DMA cannot read
                # PSUM) into the group staging tile, undoing the SH scale
                nc.scalar.activation(sig16g[:, ts(ti, NT)], psh[:], AF.Copy,
                                     scale=1.0 / SH)
                if ti == OG - 1:
                    sig16gs[g] = sig16g
                    del groups[g]

            def emit_group_xbar(g):
                # transposed epilogue, once per group: one XBAR DMA transposes
                # the staged head outputs to rows-as-partitions layout (row
                # 512t + 128c + p at xstage[p, 4t+c, :]), so sigmoid/mul/add
                # run with free-dim 16-32 instead of 512. Issued on Sync one
                # tile after the last head-psum copy, so its wait is already
                # satisfied and it never delays xt loads or fc2 relus.
                xstage = opool.tile([128, 4 * OG, 16], BF, tag="xstage")
                nc.sync.dma_start_transpose(xstage[:], sig16gs.pop(g))
                xstages[g] = xstage

            def emit_group_epilogue(g):
                gsl = ds(g * 4 * OG, 4 * OG)
                xstage = xstages.pop(g)
                nc.scalar.activation(plT[:, gsl, :], xstage[:, :, 0:2],
                                     AF.Sigmoid)
                se = spool.tile([128, 4 * OG], F32, tag="se")
                nc.vector.tensor_mul(se[:], plT[:, gsl, 1], epsT_sb[:, gsl])
                nc.vector.tensor_add(actT[:, gsl], plT[:, gsl, 0], se[:])
                nc.vector.tensor_copy(blT[:, gsl], xstage[:, :, 2])

            prep_group(0)
            for t in range(n_tiles + 2):
                if t % OG == 2 and t >= OG + 2:
                    emit_group_xbar(t // OG - 1)
                if t % OG == 0 and t >= 2 * OG:
                    emit_group_epilogue(t // OG - 2)
                if t % OG == 2 and t // OG + 1 < n_groups:
                    prep_group(t // OG + 1)
                if t < n_tiles:
                    emit_fc1(t)
                if 0 <= t - 2:
                    emit_heads(t - 2)
                if 0 <= t - 1 < n_tiles:
                    emit_fc2(t - 1)

            emit_group_xbar(n_groups - 1)
            emit_group_epilogue(n_groups - 1)
            # ship the accumulated output planes once, fully contiguous on
            # both sides, spread across both DMA-capable engines
            nc.sync.dma_start(pl_d[:], plT[:])
            nc.scalar.dma_start(act_d[:], actT[:])
            nc.sync.dma_start(bl_d[:], blT[:])

    nc.compile()
    return nc


def host_prep(frame, reward, last_action, eps, W1, b1, W2, b2, Wp, bp, Wb, bb,
              rows=R, n_cores=N_CORES):
    """Shard + lay out inputs for the device program. Returns in_maps."""
    frame = np.asarray(frame, np.float32).reshape(TB, OBS)
    reward = np.asarray(reward, np.float32).reshape(TB)
    la = np.asarray(last_action).reshape(TB).astype(FP8)
    eps = np.asarray(eps, np.float32).reshape(TB)

    W1 = np.asarray(W1, np.float32)
    W2 = np.asarray(W2, np.float32)
    b1 = np.asarray(b1, np.float32)
    b2 = np.asarray(b2, np.float32)
    Wp = np.asarray(Wp, np.float32)
    bp = np.asarray(bp, np.float32)
    Wb = np.asarray(Wb, np.float32)
    bb = np.asarray(bb, np.float32)

    # W1T scaled by S1, padded 400 -> 512 output features, fp8:
    # w1[p, b, o] = S1 * W1[o, 128b+p]
    w1t = np.zeros((512, 512), np.float32)
    w1t[:, 0:400] = S1 * W1.T
    w1_h = np.ascontiguousarray(
        w1t.reshape(4, 128, 512).transpose(1, 0, 2)).astype(FP8)
    # W2T scaled by S2/S1, padded [400->512, 300->320], fp8
    w2t = np.zeros((512, 320), np.float32)
    w2t[0:400, 0:300] = (S2 / S1) * W2.T
    w2_h = np.ascontiguousarray(
        w2t.reshape(4, 128, 320).transpose(1, 0, 2)).astype(FP8)
    # head weights (fp8, scaled by SH/S2P on h2 rows): columns 0/1/2 hold
    # (mu, sigma, baseline). Pair A planes = h2 feats 0:128 / 128:256;
    # pair B plane 0 = feats 256:320 (pad), plane 1 = [cr; la; ones]
    # weights at rows 32..34 with the ones row carrying the head biases
    # (scaled by SH since cr/la/ones are unscaled on the device)
    Wh16 = np.zeros((302, 16), np.float32)
    Wh16[:, 0] = Wp[0]
    Wh16[:, 1] = Wp[1]
    Wh16[:, 2] = Wb[0]
    wha_h = np.zeros((128, 2, 16), np.float32)
    wha_h[:, 0, :] = (SH / S2P) * Wh16[0:128]
    wha_h[:, 1, :] = (SH / S2P) * Wh16[128:256]
    wha_h = wha_h.astype(FP8)
    whb_h = np.zeros((64, 2, 16), np.float32)
    whb_h[0:44, 0, :] = (SH / S2P) * Wh16[256:300]
    whb_h[32:34, 1, :] = SH * Wh16[300:302]
    whb_h[34, 1, 0] = SH * bp[0]
    whb_h[34, 1, 1] = SH * bp[1]
    whb_h[34, 1, 2] = SH * bb[0]
    whb_h = whb_h.astype(FP8)
    b1p = np.zeros(512, np.float32)
    b1p[0:400] = S1 * b1
    b1_h = np.ascontiguousarray(b1p.reshape(4, 128).T)
    b2_h = np.zeros((128, 3), np.float32)
    b2_h[0:128, 0] = S2P * b2[0:128]
    b2_h[0:128, 1] = S2P * b2[128:256]
    b2_h[0:44, 2] = S2P * b2[256:300]

    cr = np.clip(reward, -1.0, 1.0).astype(FP8)
    ones = np.ones(rows, FP8)

    in_maps = []
    for c in range(n_cores):
        sl = slice(c * rows, (c + 1) * rows)
        xt = np.ascontiguousarray(
            frame[sl].T.reshape(4, 128, rows).transpose(1, 0, 2)).astype(FP8)
        rwla = np.stack([cr[sl], la[sl], ones], axis=0)
        # transposed row layout: row 512*t + 128*c + p sits at column 4*t + c
        # of partition p (the XBAR transpose's 128x16-tile output ordering)
        in_maps.append({
            "xt": xt,
            "rwla": rwla,
            "eps": np.ascontiguousarray(
                eps[sl].reshape(-1, 4, 128).transpose(2, 0, 1).reshape(
                    128, -1)),
            "w1": w1_h, "w2": w2_h, "wha": wha_h, "whb": whb_h,
            "b1": b1_h, "b2": b2_h,
        })
    return in_maps


def assemble_out(per_core_results):
    """Per-core dict of transposed-layout planes (row 512*t + 128*c + p at
    column 4*t + c of partition p): pl [128, nj, 2], act [128, nj],
    bl [128, nj] -> [T, B, 4]."""
    outs = []
    for r in per_core_results:
        pl = np.asarray(r["pl"]).astype(np.float32)      # [128, nj, 2]
        act = np.asarray(r["act"]).astype(np.float32)    # [128, nj]
        bl = np.asarray(r["bl"]).astype(np.float32)      # [128, nj]
        o = np.stack([pl[:, :, 0], pl[:, :, 1], bl, act], axis=-1)
        # [p, 4t+c, ch] -> rows 512t + 128c + p
        o = o.reshape(128, -1, 4, 4).transpose(1, 2, 0, 3).reshape(-1, B, 4)
        outs.append(o)
    return np.ascontiguousarray(np.concatenate(outs, axis=0))


_NC_CACHE = {}


def kernel(**inputs) -> np.ndarray:
    in_maps = host_prep(**inputs)
    if R not in _NC_CACHE:
        _NC_CACHE[R] = build_bass(R)
    nc = _NC_CACHE[R]
    res = run_bass_kernel_spmd(nc, in_maps, core_ids=list(range(N_CORES)))
    return assemble_out([res.results[c] for c in range(N_CORES)])
